# revision 1
# baseline (speedup 1.0000x reference)
"""Trainium2 Bass kernel for nn_MultiHeadAttention (B=8, S=1024, HID=1024, NH=16).

v6 over v5: rowmax via fused tensor_tensor_reduce over score halves (512
elems/pass, min-accum of negated max-pairs); eT split 2 DMA-transposes + 2
PE-transposes (PE kept warm) with copies on DVE; qT/kT bias-fold copies
alternate ACT-Identity / DVE-scalar_tensor_tensor. v5: ctx tail lagged 2 steps (never waits fresh transposes); renorm
interleaved per head-pair into the attention steps (Pool-legal tensor_tensor
add/divide, VM ones-column pre-scaled by CREN); v4 notes:
  - ALL eT transposes via dma_start_transpose (no PE transposes / engine
    copies in the attention back-half; ep psum freed -> scores bufs=3)
  - phase-1 transposes all fp32 (no DVE bf16 pre-conversions, no bf16 psum
    pool); psum->sbuf copies rotated across ACT/DVE/Pool
  - ud + ctxu extraction on Pool; renorm scale broadcast via direct
    sbuf->sbuf DMA (no DRAM roundtrip); out-proj psum from the big pool
"""

import numpy as np

import concourse.bass as bass
import concourse.tile as tile
from concourse import bacc, mybir
from concourse.bass_utils import run_bass_kernel_spmd
from concourse.masks import make_identity

F32 = mybir.dt.float32
F32R = mybir.dt.float32r
BF16 = mybir.dt.bfloat16
AF = mybir.ActivationFunctionType
ALU = mybir.AluOpType
AX = mybir.AxisListType

B, S, HID, NH, DH = 8, 1024, 1024, 16, 64
P = 128
NP = HID // P
NSB = S // P
NPAIR = NH // 2
CREN = float(S) * 1e-8

_CACHE = {}


def _build():
    nc = bacc.Bacc("TRN2", target_bir_lowering=False, debug=False, num_devices=B)

    Xq = nc.dram_tensor("Xq", [S, HID], F32, kind="ExternalInput").ap()
    Xk = nc.dram_tensor("Xk", [S, HID], F32, kind="ExternalInput").ap()
    Xv = nc.dram_tensor("Xv", [S, HID], F32, kind="ExternalInput").ap()
    Qm = nc.dram_tensor("Qm", [S], F32, kind="ExternalInput").ap()
    Km = nc.dram_tensor("Km", [S], F32, kind="ExternalInput").ap()
    Wq = nc.dram_tensor("Wq", [HID, HID], F32, kind="ExternalInput").ap()
    Wk = nc.dram_tensor("Wk", [HID, HID], F32, kind="ExternalInput").ap()
    Wv = nc.dram_tensor("Wv", [HID, HID], F32, kind="ExternalInput").ap()
    Wo = nc.dram_tensor("Wo", [HID, HID], F32, kind="ExternalInput").ap()
    bqv = nc.dram_tensor("bq", [HID], F32, kind="ExternalInput").ap()
    bkv = nc.dram_tensor("bk", [HID], F32, kind="ExternalInput").ap()
    bvv = nc.dram_tensor("bv", [HID], F32, kind="ExternalInput").ap()
    bov = nc.dram_tensor("bo", [HID], F32, kind="ExternalInput").ap()
    out = nc.dram_tensor("out", [S, HID], F32, kind="ExternalOutput").ap()
    scl_dram = nc.dram_tensor("scl_scratch", [32, 512], F32).ap()

    from contextlib import ExitStack
    with tile.TileContext(nc) as tc, ExitStack() as _es:
        def _pool(name, bufs, space=None):
            kw = {"space": space} if space else {}
            return _es.enter_context(tc.tile_pool(name=name, bufs=bufs, **kw))

        consts = _pool("consts", 1)
        xrows = _pool("xrows", 3)
        xrows16 = _pool("xrows16", 1)
        bigx = _pool("bigx", 1)
        wcp = _pool("wc", 2)
        xvcp = _pool("xvc", 1)
        qkp = _pool("qk", 1)
        vmp = _pool("vm", 1)
        etp = _pool("et", 2)
        epool = _pool("epool", 6)
        ctxp = _pool("ctx", 1)
        smalls = _pool("smalls", 4)
        stg = _pool("stg", 2)
        scdup = _pool("scdup", 1)
        ps_big = _pool("ps_big", 2, "PSUM")
        ps_sc = _pool("ps_sc", 2, "PSUM")
        ps_mid = _pool("ps_mid", 1, "PSUM")
        ps_ep = _pool("ps_ep", 1, "PSUM")
        if True:
            # ---------------- constants ----------------
            idf = consts.tile([P, P], F32, name="idf")
            make_identity(nc, idf[:])
            idb = consts.tile([P, P], BF16, name="idb")
            nc.vector.tensor_copy(idb[:], idf[:])

            bq8c = consts.tile([P, NP], F32, name="bq8c")
            nc.sync.dma_start(bq8c[:], bqv.rearrange("(o p) -> p o", p=P))
            nc.vector.tensor_scalar_mul(bq8c[:], bq8c[:], 8.0)
            bkc = consts.tile([P, NP], F32, name="bkc")
            nc.sync.dma_start(bkc[:], bkv.rearrange("(o p) -> p o", p=P))

            biasb = consts.tile([65, HID], BF16, name="biasb")  # bo@0, bv@64
            nc.gpsimd.dma_start(biasb[0:1, :], bov[None, :])
            nc.gpsimd.dma_start(biasb[64:65, :], bvv[None, :])
            onesb = consts.tile([65, P], BF16, name="onesb")
            nc.vector.memset(onesb[0:1, :], 1.0)
            nc.vector.memset(onesb[64:65, :], 1.0)

            km_pi = consts.tile([P, NSB], F32, name="km_pi")
            nc.sync.dma_start(km_pi[:], Km.rearrange("(o p) -> p o", p=P))

            # rotating psum->sbuf copy across ACT / DVE (GPSIMD can't read PSUM)
            rr = [0]

            def rot_copy(dst, src):
                k = rr[0] % 2
                rr[0] += 1
                if k == 0:
                    nc.scalar.activation(dst, src, AF.Copy)
                else:
                    nc.vector.tensor_copy(dst, src)

            # ------------- helper: transpose X -> [P, NP, S] -------------
            def build_xt(x_dram, dst):
                for sb in range(NSB):
                    xr = xrows.tile([P, HID], F32, tag="xr32")
                    nc.sync.dma_start(xr[:], x_dram[sb * P:(sb + 1) * P, :])
                    for g in range(2):
                        pt = ps_big.tile([P, 512], F32, tag="big")
                        for t in range(4):
                            ib = g * 4 + t
                            nc.tensor.transpose(pt[:, t * P:(t + 1) * P],
                                                xr[:, ib * P:(ib + 1) * P],
                                                idf[:])
                        rot_copy(dst[:, g * 4:(g + 1) * 4, sb * P:(sb + 1) * P],
                                 pt[:].rearrange("p (t c) -> p t c", t=4))

            # ------- helper: one o-block of W^T -> [P, NP, P] -------
            def build_wchunk(w_dram, ob, dtype):
                tag = "wc32" if dtype == F32R else "wc16"
                wch = wcp.tile([P, NP, P], dtype, tag=tag)
                xr = xrows.tile([P, HID], F32, tag="xr32")
                nc.sync.dma_start(xr[:], w_dram[ob * P:(ob + 1) * P, :])
                if dtype == BF16:
                    xb = xrows16.tile([P, HID], BF16, tag="xr16")
                    nc.vector.tensor_copy(xb[:], xr[:])
                    for g in range(2):
                        pt = ps_ep.tile([P, 512], BF16, tag="ep")
                        for t in range(4):
                            ib = g * 4 + t
                            nc.tensor.transpose(pt[:, t * P:(t + 1) * P],
                                                xb[:, ib * P:(ib + 1) * P],
                                                idb[:])
                        rot_copy(wch[:, g * 4:(g + 1) * 4, :],
                                 pt[:].rearrange("p (t c) -> p t c", t=4))
                else:
                    for g in range(2):
                        pt = ps_big.tile([P, 512], F32, tag="big")
                        for t in range(4):
                            ib = g * 4 + t
                            nc.tensor.transpose(pt[:, t * P:(t + 1) * P],
                                                xr[:, ib * P:(ib + 1) * P],
                                                idf[:])
                        rot_copy(wch[:, g * 4:(g + 1) * 4, :],
                                 pt[:].rearrange("p (t c) -> p t c", t=4))
                return wch

            # ---------------- q/k projections (fp32r) ----------------
            qTt = [[None, None] for _ in range(NP)]
            kTt = [[None, None] for _ in range(NP)]
            for _ob in range(NP):
                for _sc in range(2):
                    t_q = qkp.tile([P, 512], F32R, name=f"qT{_ob}_{_sc}")
                    qTt[_ob][_sc] = t_q
                    t_k = qkp.tile([P, 512], F32R, name=f"kT{_ob}_{_sc}")
                    kTt[_ob][_sc] = t_k

            def emit_qk_copy(sc, dst_ap, pp_ap, bcol, scale):
                if sc == 0:
                    nc.scalar.activation(dst_ap, pp_ap, AF.Identity,
                                         bias=bcol, scale=scale)
                else:
                    nc.vector.scalar_tensor_tensor(
                        dst_ap, pp_ap, scale,
                        bcol.to_broadcast([P, 512]), ALU.mult, ALU.add)

            def qk_projection(x_dram, w_dram, dst, bias_col, scale):
                xt_full = bigx.tile([P, NP, S], F32R, tag="bigx")
                build_xt(x_dram, xt_full)
                wch_next = build_wchunk(w_dram, 0, F32R)
                for ob in range(NP):
                    wch = wch_next
                    if ob + 1 < NP:
                        wch_next = build_wchunk(w_dram, ob + 1, F32R)
                    for sc in range(2):
                        ss = slice(sc * 512, (sc + 1) * 512)
                        pp = ps_big.tile([P, 512], F32, tag="big")
                        for m in range(NP):
                            nc.tensor.matmul(pp[:], wch[:, m, :],
                                             xt_full[:, m, ss],
                                             start=(m == 0), stop=(m == NP - 1))
                        emit_qk_copy(sc, dst[ob][sc][:, :], pp[:],
                                     bias_col[:, ob:ob + 1], scale)

            qk_projection(Xq, Wq, qTt, bq8c, 8.0)
            qk_projection(Xk, Wk, kTt, bkc, 1.0)

            # ---------------- v projection (bf16) ----------------
            VM = vmp.tile([P, NSB, NH, 66], BF16, name="VM")
            WvT = bigx.tile([P, NP, HID], BF16, tag="bigx")
            for ob in range(NP):
                wch = build_wchunk(Wv, ob, BF16)
                nc.gpsimd.tensor_copy(WvT[:, :, ob * P:(ob + 1) * P], wch[:])
            for scq in range(4):
                xvc = xvcp.tile([P, NP, 256], BF16, tag="xvc")
                for sb2 in range(2):
                    sb = scq * 2 + sb2
                    xr = xrows.tile([P, HID], F32, tag="xr32")
                    nc.sync.dma_start(xr[:], Xv[sb * P:(sb + 1) * P, :])
                    xb = xrows16.tile([P, HID], BF16, tag="xr16")
                    nc.vector.tensor_copy(xb[:], xr[:])
                    for g in range(2):
                        pt = ps_ep.tile([P, 512], BF16, tag="ep")
                        for t in range(4):
                            ib = g * 4 + t
                            nc.tensor.transpose(pt[:, t * P:(t + 1) * P],
                                                xb[:, ib * P:(ib + 1) * P],
                                                idb[:])
                        rot_copy(xvc[:, g * 4:(g + 1) * 4, sb2 * P:(sb2 + 1) * P],
                                 pt[:].rearrange("p (t c) -> p t c", t=4))
                for sb2 in range(2):
                    sb = scq * 2 + sb2
                    for oc in range(2):
                        pp = ps_big.tile([P, 512], F32, tag="big")
                        for m in range(NP):
                            nc.tensor.matmul(
                                pp[:], xvc[:, m, sb2 * P:(sb2 + 1) * P],
                                WvT[:, m, oc * 512:(oc + 1) * 512],
                                start=(m == 0), stop=False)
                        nc.tensor.matmul(pp[:], onesb[64:65, :],
                                         biasb[64:65, oc * 512:(oc + 1) * 512],
                                         start=False, stop=True)
                        nc.vector.tensor_scalar_mul(
                            VM[:, sb, oc * 8:(oc + 1) * 8, 0:64],
                            pp[:].rearrange("p (h d) -> p h d", h=8),
                            km_pi[:, sb:sb + 1])
            for sb in range(NSB):
                nc.vector.tensor_copy(
                    VM[:, sb, :, 64:65],
                    km_pi[:, sb:sb + 1, None].to_broadcast([P, NH, 1]))
            nc.vector.memset(VM[:, :, :, 65:66], CREN)

            # ---------------- WoT (bf16) in the bigx slot ----------------
            WoT = bigx.tile([P, NP, HID], BF16, tag="bigx")
            for ob in range(NP):
                wch = build_wchunk(Wo, ob, BF16)
                nc.gpsimd.tensor_copy(WoT[:, :, ob * P:(ob + 1) * P], wch[:])

            # ---------------- attention ----------------
            ST = consts.tile([32, S], F32, name="ST")  # [U | CREN*D] per row
            scl = consts.tile([32, 512], F32, name="scl")
            ctxut = []
            for j in range(NPAIR):
                cx_j = ctxp.tile([P, S], BF16, name=f"ctxu{j}")
                ctxut.append(cx_j)

            steps = [(h, qc) for h in range(NH) for qc in range(2)]
            state = {}
            etcs = {}

            qmst = consts.tile([32, 512], F32, name="qmst")
            for _idx in range(32):
                _qc = _idx % 2
                nc.sync.dma_start(qmst[_idx:_idx + 1, :],
                                  Qm[None, _qc * 512:(_qc + 1) * 512])

            def front_qb(i, t):
                h, qc = steps[i]
                j, pb = h // 2, 64 * (h % 2)
                qb = qc * 4 + t
                sp = ps_sc.tile([P, S], F32, tag="sc")
                for kc in range(2):
                    ks = slice(kc * 512, (kc + 1) * 512)
                    nc.tensor.matmul(sp[:, ks],
                                     qTt[j][qc][pb:pb + 64, t * P:(t + 1) * P],
                                     kTt[j][kc][pb:pb + 64, :],
                                     start=True, stop=True)
                nmax = smalls.tile([P, 1], F32, tag="nmax")
                nc.vector.tensor_reduce(nmax[:], sp[:], axis=AX.X,
                                        op=ALU.max, negate=True)
                e_t = epool.tile([P, S], BF16, tag="e")
                nc.scalar.activation(e_t[:], sp[:], AF.Exp,
                                     bias=nmax[:], scale=1.0)
                state.setdefault(i, []).append(e_t)

            def back_chunk(i, t):
                e_t = state[i][t]
                ep = ps_ep.tile([P, S], BF16, tag="ep")
                for kb in range(NSB):
                    nc.tensor.transpose(ep[:, kb * P:(kb + 1) * P],
                                        e_t[:, kb * P:(kb + 1) * P],
                                        idb[:])
                dst = etcs[i][:, :, t * P:(t + 1) * P]
                sap = ep[:].rearrange("p (kb q) -> p kb q", kb=NSB)
                if t % 2 == 1:
                    nc.vector.tensor_copy(dst, sap)
                else:
                    nc.scalar.activation(dst, sap, AF.Copy)

            def back_tail(i):
                h, qc = steps[i]
                j, pb = h // 2, 64 * (h % 2)
                et_c = etcs.pop(i)
                cp = ps_mid.tile([P, 512], F32, tag="mid")
                for kb in range(NSB):
                    nc.tensor.matmul(cp[0:66, :], VM[:, kb, h, :],
                                     et_c[:, kb, :],
                                     start=(kb == 0), stop=(kb == NSB - 1))
                idx = h * 2 + qc
                ud = stg.tile([2, 512], F32, tag="ud")
                nc.vector.tensor_copy(ud[:], cp[64:66, :])
                if pb == 0:
                    nc.vector.tensor_copy(
                        ctxut[j][0:64, qc * 512:(qc + 1) * 512], cp[0:64, :])
                else:
                    sg = stg.tile([64, 512], BF16, tag="sg")
                    nc.vector.tensor_copy(sg[:], cp[0:64, :])
                    nc.sync.dma_start(
                        ctxut[j][64:128, qc * 512:(qc + 1) * 512], sg[:])
                nc.sync.dma_start(ST[idx:idx + 1, 0:512], ud[0:1, :])
                nc.sync.dma_start(ST[idx:idx + 1, 512:1024], ud[1:2, :])
                del state[i]

            def renorm_pair(j):
                # scl = Qm / (U + CREN*D); ST row = [U | CREN*D]
                rows = slice(4 * j, 4 * j + 4)
                nc.gpsimd.tensor_tensor(scl[rows, :], ST[rows, 0:512],
                                        ST[rows, 512:1024], ALU.add)
                nc.gpsimd.tensor_tensor(scl[rows, :], qmst[rows, :],
                                        scl[rows, :], ALU.divide)
                for qc in range(2):
                    sd = scdup.tile([P, 512], F32, tag="sd")
                    ia = 4 * j + qc
                    nc.gpsimd.dma_start(
                        sd[:].rearrange("(two p) f -> two p f", two=2),
                        scl[ia:ia + 3:2, None, :].to_broadcast([2, 64, 512]))
                    nc.gpsimd.tensor_tensor(
                        ctxut[j][:, qc * 512:(qc + 1) * 512],
                        ctxut[j][:, qc * 512:(qc + 1) * 512], sd[:], ALU.mult)

            nsteps = len(steps)
            for t in range(4):
                front_qb(0, t)
            for i in range(1, nsteps):
                et_new = etp.tile([P, NSB, 512], BF16, tag="etc")
                etcs[i - 1] = et_new
                for t in range(4):
                    front_qb(i, t)
                    back_chunk(i - 1, t)
                if i >= 2:
                    back_tail(i - 2)
            et_new = etp.tile([P, NSB, 512], BF16, tag="etc")
            etcs[nsteps - 1] = et_new
            back_tail(nsteps - 2)
            for t in range(4):
                back_chunk(nsteps - 1, t)
            back_tail(nsteps - 1)

            # ---------------- batched renorm (baseline pattern) ----------------
            # scl = Qm / (U + CREN * D);  U = ST[:, 0:512], D' = CREN*D = ST[:, 512:]
            nc.vector.tensor_tensor(scl[:], ST[:, 0:512], ST[:, 512:1024],
                                    ALU.add)
            nc.vector.reciprocal(scl[:], scl[:])
            nc.vector.tensor_tensor(scl[:], scl[:], qmst[:], ALU.mult)
            nc.sync.dma_start(scl_dram[:], scl[:])
            for j in range(NPAIR):
                for qc in range(2):
                    sd = scdup.tile([P, 512], F32, tag="sd")
                    ia = 4 * j + qc
                    ibx = 4 * j + 2 + qc
                    nc.sync.dma_start(
                        sd[0:64, :],
                        scl_dram[ia:ia + 1, :].to_broadcast([64, 512]))
                    nc.sync.dma_start(
                        sd[64:128, :],
                        scl_dram[ibx:ibx + 1, :].to_broadcast([64, 512]))
                    eng = nc.vector if (2 * j + qc) % 2 else nc.gpsimd
                    eng.tensor_tensor(
                        ctxut[j][:, qc * 512:(qc + 1) * 512],
                        ctxut[j][:, qc * 512:(qc + 1) * 512], sd[:], ALU.mult)

            # ---------------- output projection ----------------
            for qb in range(NSB):
                for oc in range(2):
                    op_ = ps_big.tile([P, 512], F32, tag="big")
                    for j in range(NPAIR):
                        nc.tensor.matmul(
                            op_[:], ctxut[j][:, qb * P:(qb + 1) * P],
                            WoT[:, j, oc * 512:(oc + 1) * 512],
                            start=(j == 0), stop=False)
                    nc.tensor.matmul(op_[:], onesb[0:1, :],
                                     biasb[0:1, oc * 512:(oc + 1) * 512],
                                     start=False, stop=True)
                    ot = scdup.tile([P, 512], F32, tag="sd")
                    rot_copy(ot[:], op_[:])
                    nc.sync.dma_start(
                        out[qb * P:(qb + 1) * P, oc * 512:(oc + 1) * 512], ot[:])

    nc.compile()
    return nc


def kernel(Q, K, V, Q_mask, K_mask, Wq, bq, Wk, bk, Wv, bv, Wo, bo):
    if "nc" not in _CACHE:
        _CACHE["nc"] = _build()
    nc = _CACHE["nc"]
    Q = np.ascontiguousarray(np.asarray(Q, np.float32))
    K = np.ascontiguousarray(np.asarray(K, np.float32))
    V = np.ascontiguousarray(np.asarray(V, np.float32))
    shared = {
        "Wq": np.ascontiguousarray(np.asarray(Wq, np.float32)),
        "Wk": np.ascontiguousarray(np.asarray(Wk, np.float32)),
        "Wv": np.ascontiguousarray(np.asarray(Wv, np.float32)),
        "Wo": np.ascontiguousarray(np.asarray(Wo, np.float32)),
        "bq": np.ascontiguousarray(np.asarray(bq, np.float32)),
        "bk": np.ascontiguousarray(np.asarray(bk, np.float32)),
        "bv": np.ascontiguousarray(np.asarray(bv, np.float32)),
        "bo": np.ascontiguousarray(np.asarray(bo, np.float32)),
    }
    in_maps = []
    for i in range(B):
        m = dict(shared)
        m["Xq"] = np.ascontiguousarray(Q[i])
        m["Xk"] = np.ascontiguousarray(K[i])
        m["Xv"] = np.ascontiguousarray(V[i])
        m["Qm"] = np.ascontiguousarray(np.asarray(Q_mask[i], np.float32))
        m["Km"] = np.ascontiguousarray(np.asarray(K_mask[i], np.float32))
        in_maps.append(m)
    res = run_bass_kernel_spmd(nc, in_maps, list(range(B)))
    return np.stack([res.results[i]["out"] for i in range(B)], axis=0)

